# revision 8
# baseline (speedup 1.0000x reference)
"""Trainium2 Bass kernel for masked additive-attention pooling.

Reference math (per batch b):
    whhn = encoding @ W_h.T                            # [B, D]
    M    = tanh(X @ W_y.T + whhn[:, None, :])          # [B, T, D]
    a    = sigmoid(M @ w_a)                            # [B, T]
    e    = exp(a); den = sum(e * mask); w = e * mask / den
    out  = sum_t w[t] * X[t]                           # [B, D]

Sharding: data-parallel over batch B=32 across 8 cores (4 batch slots
per core). Weights replicated. Host does layout transforms only.

Length-aware work skipping: the mask is a prefix mask, so each batch
only needs ceil(len/128) 128-token chunks of compute. Batches are
sorted by length and dealt into 8 cores x 4 slots; one SPMD program
with per-slot chunk counts template[s] = ceil(max_len_in_slot/128)
serves all cores; shorter batches are zero-padded (pad chunks have
mask 0). The template is derived from the mask at runtime; programs
are compiled per template and cached.

Device strategy per core:
  - X^T arrives host-pretransposed in fp8; z = Wy.X^T as fp8 DoubleRow
    matmuls over 512-token tiles (157 TF/s peak); the 1/32 weight
    scale and the per-(e,slot) whhn bias fold into the tanh ACT.
  - logits flipped vs the naive form: w_a (padded to the M=32 minimum
    for dual-fp8 LDWEIGHTS) is the stationary operand, tanh output
    streams through -> a_pre rows [1, ntile] in PSUM, avoiding
    reloading all of tanh(z) as PE weights. Rows are transposed back
    to token-on-partition columns OFF the tensor engine: gpsimd
    partition_broadcast + affine_select diagonal extract + vector
    reduce.
  - sigmoid/exp run on [128, nch] column tiles (full-lane ACT);
    sigmoid via 0.5 + 0.5*tanh(x/2) so tanh and exp share one
    activation table set -> single ACT_TABLE_LOAD for the kernel.
  - denominator via vector free-reduce + one gpsimd partition
    all-reduce per slot (no matmuls).
  - pooling num as bf16 matmuls against bf16 X tiles, fp32 PSUM
    accumulation per slot; one reciprocal + scale at each slot end.
    (fp8 X for pooling was tried: rel err 3.4e-2 > the 2e-2 gate.)
  - the per-tile epilogue is sliced into micro-tasks interleaved
    between the next tile's z matmul groups so the PE stays busy.
  - NOTE: the kernel is close to chip-HBM-bound: ~22 MB/core in,
    8 cores share the device HBM (~138 GB/s/core effective measured),
    so tensor-stream savings beyond this mostly turn into DMA waits.
"""

import sys

if "/opt/trn_rl_repo" not in sys.path:
    sys.path.insert(0, "/opt/trn_rl_repo")

import numpy as np
import ml_dtypes

import concourse.bacc as bacc
import concourse.mybir as mybir
import concourse.tile as tile
import concourse.bass_isa as bass_isa
from concourse.bass_utils import run_bass_kernel_spmd

F32 = mybir.dt.float32
BF16 = mybir.dt.bfloat16
FP8 = mybir.dt.float8e4
AF = mybir.ActivationFunctionType
DR = mybir.MatmulPerfMode.DoubleRow
AX = mybir.AxisListType
ALU = mybir.AluOpType

N_CORES = 8
B, T, D = 32, 2048, 1024
B_LOC = B // N_CORES          # 4 batch slots per core
KD = D // 128                 # 8 contraction chunks
EB = D // 128                 # 8 output-feature blocks
TILE_CH = 4                   # chunks per z tile (512 tokens)
Z_LDW = False                 # standalone DR-fp8 ldweights is ISA-illegal

_CACHE = {}


def _tiles_of(template):
    tiles = []
    choff = 0
    for s, tch in enumerate(template):
        groups = [TILE_CH] * (tch // TILE_CH)
        if tch % TILE_CH:
            groups.append(tch % TILE_CH)
        for gi, g in enumerate(groups):
            tiles.append(dict(s=s, nch=g, choff=choff,
                              col0=choff - sum(template[:s]),
                              first=(gi == 0), last=(gi == len(groups) - 1)))
            choff += g
    return tiles


def _halves(nch):
    n = nch * 128
    if n > 512:
        return [(0, 512), (512, n - 512)]
    return [(0, n)]


def build(template):
    nch_tot = sum(template)
    ntokv = nch_tot * 128
    tiles = _tiles_of(template)
    nt = len(tiles)
    slot_first_tile = {}
    for j, tl in enumerate(tiles):
        if tl["first"]:
            slot_first_tile[tl["s"]] = j

    nc = bacc.Bacc("TRN2", target_bir_lowering=False, debug=False,
                   num_devices=N_CORES)

    xn_d = nc.dram_tensor("xn", [128, nch_tot * D], BF16,
                          kind="ExternalInput").ap()
    xt_d = nc.dram_tensor("xt", [128, KD * ntokv], FP8,
                          kind="ExternalInput").ap()
    wyt = nc.dram_tensor("wyt", [EB, 128, KD * 128], FP8,
                         kind="ExternalInput").ap()
    wht = nc.dram_tensor("wht", [EB, 128, KD * 128], BF16,
                         kind="ExternalInput").ap()
    enc_cols = nc.dram_tensor("enc_cols", [128, KD * B_LOC], BF16,
                              kind="ExternalInput").ap()
    # wa padded to M=32 stationary columns (dual-fp8 LDW needs M>=32);
    # layout [p, (q, i, m32)], only m=0 is real weight, rest zero
    wa_cols = nc.dram_tensor("wa_cols", [128, 4 * 2 * 32], FP8,
                             kind="ExternalInput").ap()
    mask_cols = nc.dram_tensor("mask_cols", [128, nch_tot], F32,
                               kind="ExternalInput").ap()
    out = nc.dram_tensor("out", [B_LOC, D], F32, kind="ExternalOutput").ap()

    with tile.TileContext(nc) as tc:
        with tc.tile_pool(name="consts", bufs=1) as cp, \
             tc.tile_pool(name="wy", bufs=1) as wyp, \
             tc.tile_pool(name="xnat", bufs=5) as xp, \
             tc.tile_pool(name="xt", bufs=3) as xtp, \
             tc.tile_pool(name="th", bufs=3) as thp, \
             tc.tile_pool(name="small", bufs=3) as smp, \
             tc.tile_pool(name="gp", bufs=2) as gp, \
             tc.tile_pool(name="mps", bufs=1, space="PSUM") as psum:

            state = {}

            def load_xt(j, split=1):
                tl = tiles[j]
                wfull = KD * tl["nch"] * 128
                t = xtp.tile([128, KD * TILE_CH * 128], FP8, tag="xt",
                             name=f"xt_{j}")
                w = wfull // split
                o = KD * tl["choff"] * 128
                for s in range(split):
                    nc.sync.dma_start(
                        t[:, s * w:(s + 1) * w],
                        xt_d[:, o + s * w:o + (s + 1) * w])
                state[("xt", j)] = t

            def load_xnat(j):
                tl = tiles[j]
                t = xp.tile([128, TILE_CH * D], BF16, tag="xn",
                            name=f"x_{j}")
                nc.sync.dma_start(
                    t[:, :tl["nch"] * D],
                    xn_d[:, tl["choff"] * D:(tl["choff"] + tl["nch"]) * D])
                state[("xn", j)] = t

            # ---- phase 0: weights + constants ----
            # DMA order matters at startup: tile-0 fp8 data + first wy
            # blocks unblock z; wh+enc unblock whhn (needed before any
            # tanh), so they go before the later wy blocks.
            load_xt(0, split=8)
            wy_sb = []
            for eb in range(4):
                t = wyp.tile([128, KD * 128], FP8, tag=f"wy{eb}")
                nc.sync.dma_start(t[:], wyt[eb])
                wy_sb.append(t)
            enc_sb = cp.tile([128, KD * B_LOC], BF16)
            nc.sync.dma_start(enc_sb[:], enc_cols[:])
            wh_sb = []
            whp_cm = tc.tile_pool(name="wh", bufs=1)
            whp = whp_cm.__enter__()
            for eb in range(EB):
                t = whp.tile([128, KD * 128], BF16, tag=f"wh{eb}")
                nc.sync.dma_start(t[:], wht[eb])
                wh_sb.append(t)
            for eb in range(4, EB):
                t = wyp.tile([128, KD * 128], FP8, tag=f"wy{eb}")
                nc.sync.dma_start(t[:], wyt[eb])
                wy_sb.append(t)
            if nt > 1:
                load_xt(1)
            half_sb = cp.tile([128, 1], F32)
            nc.vector.memset(half_sb[:], 0.5)
            one_sb = cp.tile([1, 1], F32)
            nc.vector.memset(one_sb[:], 1.0)
            wa_sb = cp.tile([128, 4 * 2 * 32], FP8)
            mask_sb = cp.tile([128, nch_tot], F32)
            whhn_sb = cp.tile([128, EB * B_LOC], F32)
            nc.sync.dma_start(wa_sb[:], wa_cols[:])
            nc.sync.dma_start(mask_sb[:], mask_cols[:])
            load_xnat(0)

            def emit_whhn():
                for eb in range(EB):
                    php = psum.tile([128, B_LOC], F32, tag="num", bufs=2,
                                    name=f"php_{eb}")
                    for k in range(KD):
                        nc.tensor.matmul(
                            php[:], wh_sb[eb][:, k * 128:(k + 1) * 128],
                            enc_sb[:, k * B_LOC:(k + 1) * B_LOC],
                            start=(k == 0), stop=(k == KD - 1))
                    nc.vector.tensor_copy(
                        whhn_sb[:, eb * B_LOC:(eb + 1) * B_LOC], php[:])

            # ---- main loop ----
            def emit_z_mm(j, eb):
                tl = tiles[j]
                n = tl["nch"] * 128
                hs = _halves(tl["nch"])
                zps = []
                xtv = (state[("xt", j)][:, :KD * n]
                       .rearrange("p (k n) -> p k n", k=KD))
                # half-outer, q-inner: each accumulation group stays in
                # one PSUM bank (bank alternation between consecutive
                # matmuls costs ~50ns/pass)
                for hi, (h0, nh) in enumerate(hs):
                    zp = psum.tile([128, 512], F32, tag="z", bufs=4,
                                   name=f"z_{j}_{eb}_{hi}")
                    for q in range(KD // 2):
                        if q == 2 and pending:
                            pending.pop(0)()
                        wv = (wy_sb[eb][:, q * 256:(q + 1) * 256]
                              .rearrange("p (i m) -> p i m", i=2))
                        nc.tensor.matmul(
                            zp[:, :nh], wv, xtv[:, 2 * q:2 * q + 2,
                                                h0:h0 + nh],
                            start=(q == 0), stop=(q == KD // 2 - 1),
                            perf_mode=DR)
                    zps.append(zp)
                return zps

            def emit_tanh(j, eb, zps):
                tl = tiles[j]
                s = tl["s"]
                n = tl["nch"] * 128
                pr = eb // 2
                if eb % 2 == 0:
                    th_t = thp.tile([128, 2 * TILE_CH * 128], FP8,
                                    tag=f"th{pr}", name=f"th_{j}_{pr}")
                    state[("th", j, pr)] = th_t
                else:
                    th_t = state[("th", j, pr)]
                for (h0, nh), zp in zip(_halves(tl["nch"]), zps):
                    nc.scalar.activation(
                        th_t[:, (eb % 2) * n + h0:(eb % 2) * n + h0 + nh],
                        zp[:, :nh], AF.Tanh, scale=1.0 / 32.0,
                        bias=whhn_sb[:, eb * B_LOC + s:eb * B_LOC + s + 1])

            def emit_z(j, ebs):
                for eb in ebs:
                    emit_tanh(j, eb, emit_z_mm(j, eb))

            pending = []

            def get_apc(s):
                if ("apc", s) not in state:
                    state[("apc", s)] = smp.tile(
                        [128, max(template)], F32, tag="apc", bufs=2,
                        name=f"apc_{s}")
                return state[("apc", s)]

            def queue_apre(j2):
                tl = tiles[j2]
                n = tl["nch"] * 128
                hs = _halves(tl["nch"])

                def apre_mm():
                    aps = []
                    for hi, (h0, nh) in enumerate(hs):
                        ap = psum.tile([32, 512], F32, tag="scr", bufs=2,
                                       name=f"ap_{j2}_{hi}")
                        for q in range(4):
                            wv = (wa_sb[:, q * 64:(q + 1) * 64]
                                  .rearrange("p (i m) -> p i m", i=2))
                            th3 = (state[("th", j2, q)][:, :2 * n]
                                   .rearrange("p (i n) -> p i n", i=2))
                            nc.tensor.matmul(
                                ap[:, :nh], wv, th3[:, :, h0:h0 + nh],
                                start=(q == 0), stop=(q == 3),
                                perf_mode=DR)
                        aps.append(ap)
                    state[("aps", j2)] = aps
                    for q in range(4):
                        state.pop(("th", j2, q))

                def bcast():
                    aps = state.pop(("aps", j2))
                    arow = smp.tile([1, TILE_CH * 128], BF16, tag="arow",
                                    bufs=2, name=f"arow_{j2}")
                    for (h0, nh), ap in zip(hs, aps):
                        nc.vector.tensor_copy(arow[:, h0:h0 + nh],
                                              ap[0:1, :nh])
                    bc = gp.tile([128, TILE_CH * 128], BF16, tag="bc",
                                 name=f"bc_{j2}")
                    nc.gpsimd.partition_broadcast(bc[:, :n], arow[:, :n])
                    state[("bc", j2)] = bc

                def to_row():
                    # last-slot variant: row copy only; the transpose
                    # runs on the (tail-idle) PE instead of gpsimd
                    aps = state.pop(("aps", j2))
                    arow = smp.tile([1, TILE_CH * 128], F32, tag="arow",
                                    bufs=2, name=f"arow_{j2}")
                    for (h0, nh), ap in zip(hs, aps):
                        nc.vector.tensor_copy(arow[:, h0:h0 + nh],
                                              ap[0:1, :nh])
                    state[("arow", j2)] = arow

                def to_cols():
                    arow = state.pop(("arow", j2))
                    apt = psum.tile([128, TILE_CH], F32, tag="scr",
                                    bufs=2, name=f"apt_{j2}")
                    for c in range(tl["nch"]):
                        nc.tensor.transpose(
                            apt[:, c:c + 1],
                            arow[:, c * 128:(c + 1) * 128], one_sb[:])
                    apc = get_apc(tl["s"])
                    nc.vector.tensor_copy(
                        apc[:, tl["col0"]:tl["col0"] + tl["nch"]],
                        apt[:, :tl["nch"]])

                def diag():
                    bc = state.pop(("bc", j2))
                    sel = gp.tile([128, TILE_CH * 128], BF16, tag="sel",
                                  name=f"sel_{j2}")
                    nc.gpsimd.affine_select(
                        sel[:, :n].rearrange("p (c t) -> p c t", t=128),
                        bc[:, :n].rearrange("p (c t) -> p c t", t=128),
                        pattern=[[0, tl["nch"]], [1, 128]],
                        compare_op=ALU.is_equal,
                        fill=0.0, base=0, channel_multiplier=-1)
                    apc = get_apc(tl["s"])
                    nc.vector.tensor_reduce(
                        apc[:, tl["col0"]:tl["col0"] + tl["nch"]],
                        sel[:, :n].rearrange("p (c t) -> p c t", t=128),
                        axis=AX.X, op=ALU.add)

                pending.append(apre_mm)
                if tl["s"] >= B_LOC - 2:
                    pending.append(to_row)
                    pending.append(to_cols)
                else:
                    pending.append(bcast)
                    pending.append(diag)

            def queue_epilogue(s):
                nch_s = template[s]
                sl0 = sum(template[:s])

                def act_chain():
                    apc = state.pop(("apc", s))
                    # sigmoid(x) = 0.5 + 0.5*tanh(x/2); tanh and exp share
                    # one table set (exp_and_others) -> no table reloads.
                    tcol = smp.tile([128, nch_s], F32, tag="tcol",
                                    name=f"tcol_{s}")
                    nc.scalar.activation(tcol[:], apc[:, :nch_s], AF.Tanh,
                                         scale=0.5 / 32.0)
                    ex = smp.tile([128, nch_s], F32, tag="ex",
                                  name=f"ex_{s}")
                    nc.scalar.activation(ex[:], tcol[:], AF.Exp,
                                         bias=half_sb[:], scale=0.5)
                    ew = smp.tile([128, nch_s], BF16, tag="ew",
                                  name=f"ew_{s}")
                    nc.vector.tensor_mul(
                        ew[:], ex[:], mask_sb[:, sl0:sl0 + nch_s])
                    state[("ew", s)] = ew
                    denp = smp.tile([128, 1], F32, tag="denp",
                                    name=f"denp_{s}")
                    nc.vector.tensor_reduce(denp[:], ew[:], axis=AX.X,
                                            op=ALU.add)
                    den = smp.tile([128, 1], F32, tag="den", bufs=2,
                                   name=f"den_{s}")
                    nc.gpsimd.partition_all_reduce(
                        den[:], denp[:], channels=128,
                        reduce_op=bass_isa.ReduceOp.add)
                    state[("den", s)] = den
                    state[("num", s)] = [
                        psum.tile([1, 512], F32, tag="num", bufs=2,
                                  name=f"num_{s}_{dn}") for dn in range(2)]

                def pool_cols(cols):
                    def fn():
                        ew = state[("ew", s)]
                        num = state[("num", s)]
                        for col in cols:
                            jj, c = col // TILE_CH, col % TILE_CH
                            xn = state[("xn", slot_first_tile[s] + jj)]
                            st = col == 0
                            sp = col == nch_s - 1
                            for dn in range(2):
                                nc.tensor.matmul(
                                    num[dn][:], ew[:, col:col + 1],
                                    xn[:, c * D + dn * 512:
                                       c * D + (dn + 1) * 512],
                                    start=st, stop=sp)
                    return fn

                def finish():
                    jj = slot_first_tile[s]
                    while jj < nt and tiles[jj]["s"] == s:
                        state.pop(("xn", jj))
                        jj += 1
                    state.pop(("ew", s))
                    num = state.pop(("num", s))
                    den = state.pop(("den", s))
                    rec = smp.tile([1, 1], F32, tag="rec", name=f"rec_{s}")
                    nc.vector.reciprocal(rec[:], den[0:1, :])
                    ob = smp.tile([1, D], F32, tag="ob", bufs=2,
                                  name=f"ob_{s}")
                    for dn in range(2):
                        nc.vector.tensor_scalar_mul(
                            ob[:, dn * 512:(dn + 1) * 512],
                            num[dn][:], rec[:])
                    nc.sync.dma_start(out[s:s + 1, :], ob[:])

                pending.append(act_chain)
                for c0 in range(0, nch_s, 2):
                    pending.append(pool_cols(range(c0, min(c0 + 2, nch_s))))
                pending.append(finish)

            for j in range(nt):
                if j + 2 < nt:
                    load_xt(j + 2)
                if j + 1 < nt:
                    load_xnat(j + 1)
                if j == 0:
                    zps01 = [emit_z_mm(0, eb) for eb in range(2)]
                    emit_whhn()
                    whp_cm.__exit__(None, None, None)
                    for eb in range(2):
                        emit_tanh(0, eb, zps01[eb])
                    emit_z(0, range(2, 4))
                else:
                    emit_z(j, range(0, 4))
                if j > 0:
                    queue_apre(j - 1)
                if tiles[j]["first"] and j > 0:
                    queue_epilogue(tiles[j - 1]["s"])
                emit_z(j, range(4, 8))
            queue_apre(nt - 1)
            queue_epilogue(tiles[nt - 1]["s"])
            while pending:
                pending.pop(0)()

    nc.compile()
    return nc


def _plan(mask):
    lengths = mask.sum(axis=1).astype(np.int64)
    t = mask.shape[1]
    prefix = (np.arange(t)[None, :] < lengths[:, None]).astype(mask.dtype)
    if not np.array_equal(prefix, mask):
        lengths = np.full(mask.shape[0], t, dtype=np.int64)
    order = np.argsort(-lengths, kind="stable")
    nch = np.maximum((lengths + 127) // 128, 1)
    template = tuple(int(nch[order[s * N_CORES]]) for s in range(B_LOC))
    return template, order, nch


def _host_pack(template, order, nch, full_input, encoding, mask,
               W_h, W_y, w_a):
    nch_tot = sum(template)
    ntokv = nch_tot * 128
    tiles = _tiles_of(template)
    wyT = np.ascontiguousarray(W_y.T)
    whT = np.ascontiguousarray(W_h.T)
    wyt_rows = np.empty((EB, 128, KD * 128), ml_dtypes.float8_e4m3)
    wht_rows = np.empty((EB, 128, KD * 128), ml_dtypes.bfloat16)
    for eb in range(EB):
        for k in range(KD):
            wyt_rows[eb, :, k * 128:(k + 1) * 128] = (
                32.0 * wyT[k * 128:(k + 1) * 128, eb * 128:(eb + 1) * 128])
            wht_rows[eb, :, k * 128:(k + 1) * 128] = \
                whT[k * 128:(k + 1) * 128, eb * 128:(eb + 1) * 128]
    wa_c = np.zeros((128, 4, 2, 32), np.float32)
    wa_cols_v = 32.0 * w_a.reshape(KD, 128).T    # [p, eb]
    for q in range(4):
        for i in range(2):
            wa_c[:, q, i, 0] = wa_cols_v[:, 2 * q + i]
    wa_c = np.ascontiguousarray(
        wa_c.reshape(128, 256)).astype(ml_dtypes.float8_e4m3)

    in_maps = []
    for i in range(N_CORES):
        xp = np.zeros((ntokv, D), np.float32)
        mcols = np.zeros((128, nch_tot), np.float32)
        enc_i = np.empty((B_LOC, D), np.float32)
        choff = 0
        for s in range(B_LOC):
            b = int(order[s * N_CORES + i])
            kept = int(min(nch[b], template[s])) * 128
            kept = min(kept, T)
            xp[choff * 128:choff * 128 + kept] = full_input[b, :kept]
            mcols[:, choff:choff + kept // 128] = \
                mask[b, :kept].reshape(kept // 128, 128).T
            enc_i[s] = encoding[b]
            choff += template[s]
        x_i = np.ascontiguousarray(
            xp.reshape(nch_tot, 128, D).transpose(1, 0, 2)
            .reshape(128, nch_tot * D)).astype(ml_dtypes.bfloat16)
        xpt = xp.T.astype(ml_dtypes.float8_e4m3).reshape(KD, 128, ntokv)
        xt_i = np.empty((128, KD * ntokv), ml_dtypes.float8_e4m3)
        for tl in tiles:
            t0, n = tl["choff"] * 128, tl["nch"] * 128
            xt_i[:, KD * t0:KD * (t0 + n)] = (
                xpt[:, :, t0:t0 + n].transpose(1, 0, 2).reshape(128, KD * n))
        enc_c = np.ascontiguousarray(
            enc_i.T.reshape(KD, 128, B_LOC).transpose(1, 0, 2)
            .reshape(128, KD * B_LOC)).astype(ml_dtypes.bfloat16)
        in_maps.append({
            "xn": x_i, "xt": xt_i, "wyt": wyt_rows, "wht": wht_rows,
            "enc_cols": enc_c, "wa_cols": wa_c, "mask_cols": mcols,
        })
    return in_maps


def run(inputs, trace=False):
    template, order, nch = _plan(np.asarray(inputs["mask"]))
    if template not in _CACHE:
        _CACHE[template] = build(template)
    nc = _CACHE[template]
    in_maps = _host_pack(template, order, nch, **inputs)
    res = run_bass_kernel_spmd(nc, in_maps, core_ids=list(range(N_CORES)),
                               trace=trace)
    out = np.empty((B, D), np.float32)
    for i in range(N_CORES):
        for s in range(B_LOC):
            out[int(order[s * N_CORES + i])] = res.results[i]["out"][s]
    return out, res


def kernel(**inputs):
    inputs = {k: np.asarray(v) for k, v in inputs.items()}
    out, _ = run(inputs, trace=False)
    return out
